# revision 1
# baseline (speedup 1.0000x reference)
"""Trainium2 Bass kernel for nn_DecoderRNN (LSTM decoder with tag-conditioned
inputs, packed-sequence output projection).

Strategy (8 NeuronCores, SPMD single program, data-driven sharding):
  - Embedding gather + input-projection gx: token-sharded with INTERLEAVED
    timestep blocks (core c owns steps {c, c+8, c+16, c+24}), distributed by
    four pipelined AllGathers so the scan starts after the first one.
  - LSTM recurrence: replicated full-batch on every core (per-step cross-core
    h exchange is latency-prohibitive: ~14us/AllGather).
  - Output projection: vocab-sharded; its matmuls are interleaved into the
    scan as packed-row tiles complete, keeping TensorE dense and HAM-warm.
Compute dtype: fp16 operands into the PE (full rate), fp32 accumulation.
"""

import sys

sys.path.insert(0, "/opt/trn_rl_repo")

import numpy as np

import concourse.bass as bass
import concourse.mybir as mybir
import concourse.tile as tile
from concourse import bacc
from concourse.bass import ts
from concourse.bass_utils import run_bass_kernel_spmd
from concourse.masks import make_identity

B, L, E, H, V, TAG = 128, 31, 512, 1024, 30000, 512
T = L + 1
NC = 8
VS = V // NC          # vocab shard per core
TB = T // NC          # timestep blocks per core
G4 = 4 * H            # gate width
F16 = mybir.dt.float16
F32 = mybir.dt.float32
AF = mybir.ActivationFunctionType
NCH = (VS + 511) // 512  # projection vocab chunks per core


def _build(n_t, off_t, p_pad):
    nc = bacc.Bacc(None, target_bir_lowering=False)

    emb_tab = nc.declare_dram_parameter("emb_tab", [V + B, E], F16, isOutput=False)
    idx_in = nc.declare_dram_parameter("idx", [B, TB], mybir.dt.int32, isOutput=False)
    wihx = nc.declare_dram_parameter("wihx", [E, G4], F16, isOutput=False)
    wiht = nc.declare_dram_parameter("wiht", [5 * 128, G4], F16, isOutput=False)
    tags_t = nc.declare_dram_parameter("tags_t", [5 * 128, B], F16, isOutput=False)
    whh = nc.declare_dram_parameter("whh", [H, G4], F16, isOutput=False)
    wlin = nc.declare_dram_parameter("wlin", [H, VS], F16, isOutput=False)
    blin = nc.declare_dram_parameter("blin", [1, VS], F16, isOutput=False)
    out = nc.declare_dram_parameter("out", [p_pad, VS], F32, isOutput=True)

    m_tiles = p_pad // 128
    gate_order = [2, 0, 1, 3]  # tanh(g) first, sigmoid(o) last
    gate_fn = {0: AF.Sigmoid, 1: AF.Sigmoid, 2: AF.Tanh, 3: AF.Sigmoid}

    from contextlib import ExitStack

    with tile.TileContext(nc) as tc:
        stack = ExitStack()
        with stack:
            const = stack.enter_context(tc.tile_pool(name="const", bufs=1))
            work = stack.enter_context(tc.tile_pool(name="work", bufs=3))
            gates = stack.enter_context(tc.tile_pool(name="gates", bufs=1))
            stmp = stack.enter_context(tc.tile_pool(name="scan_tmp", bufs=1))
            gxp = stack.enter_context(tc.tile_pool(name="gxb", bufs=3))
            lhsp = stack.enter_context(tc.tile_pool(name="lhs_proj", bufs=3))
            ostage = stack.enter_context(tc.tile_pool(name="ostage", bufs=2))
            psA = stack.enter_context(tc.tile_pool(name="psA", bufs=3, space="PSUM"))
            psT = stack.enter_context(tc.tile_pool(name="psT", bufs=2, space="PSUM"))
            dram = stack.enter_context(tc.tile_pool(name="dram", bufs=1, space="DRAM"))
            phase_stack = ExitStack()
            pha = phase_stack.enter_context(tc.tile_pool(name="phase_a", bufs=1))
            wstr = phase_stack.enter_context(tc.tile_pool(name="wstream", bufs=3))
            # ---- dummy collective first: absorbs ncfw warmup during phase A
            d_in = dram.tile([1, 128], F32)
            d_out = dram.tile([NC, 128], F32)
            d_in_sb = const.tile([1, 128], F32)
            nc.vector.memset(d_in_sb[:], 0.0)
            nc.sync.dma_start(out=d_in[:], in_=d_in_sb[:])
            nc.gpsimd.collective_compute(
                "AllGather",
                mybir.AluOpType.bypass,
                replica_groups=[list(range(NC))],
                ins=[d_in[:].opt()],
                outs=[d_out[:].opt()],
            )

            ident = const.tile([128, 128], F16)
            make_identity(nc, ident)

            # ---- phase A: gather + transpose x, tb = tags@Wiht^T + b
            idx_sb = const.tile([B, TB], mybir.dt.int32)
            nc.sync.dma_start(out=idx_sb[:], in_=idx_in[:, :])

            # all gathers + big phase-A loads issued up-front
            tags_sb = pha.tile([128, 5, B], F16)
            nc.sync.dma_start(out=tags_sb[:], in_=tags_t.ap().rearrange("(k p) b -> p k b", p=128))
            wihx_sb = pha.tile([128, 4, G4], F16)
            nc.sync.dma_start(out=wihx_sb[:], in_=wihx.ap().rearrange("(k p) n -> p k n", p=128))
            gtiles = []
            for tau in range(TB):
                g = work.tile([B, E], F16, tag=f"gather{tau}")
                nc.gpsimd.indirect_dma_start(
                    out=g[:],
                    out_offset=None,
                    in_=emb_tab[:],
                    in_offset=bass.IndirectOffsetOnAxis(ap=idx_sb[:, tau : tau + 1], axis=0),
                )
                gtiles.append(g)
            xT = pha.tile([128, TB, 4, 128], F16)  # [p, tau, kE, token]
            for tau in range(TB):
                for k in range(4):
                    pt = psT.tile([128, 128], F16, space="PSUM")
                    nc.tensor.transpose(pt[:], gtiles[tau][:, ts(k, 128)], ident[:])
                    nc.vector.tensor_copy(out=xT[:, tau, k, :], in_=pt[:])

            tb_sb = pha.tile([B, G4], F16)
            for n in range(8):
                w = wstr.tile([128, 8, 512], F16, tag="wstream")
                nc.sync.dma_start(
                    out=w[:, :5, :],
                    in_=wiht.ap()[:, ts(n, 512)].rearrange("(k p) n -> p k n", p=128),
                )
                ps = psA.tile([128, 512], F32, space="PSUM")
                for k in range(5):
                    nc.tensor.matmul(
                        out=ps[:B, :],
                        lhsT=tags_sb[:, k, :],
                        rhs=w[:, k, :],
                        start=(k == 0),
                        stop=(k == 4),
                    )
                nc.vector.tensor_copy(out=tb_sb[:, ts(n, 512)], in_=ps[:B, :])

            # ---- gx shard (tau-outer so each AllGather fires early)
            gx_in = [dram.tile([B, G4], F16, name=f"gx_in{tau}") for tau in range(TB)]
            ag_out = [dram.tile([NC, B, G4], F16, name=f"ag_out{tau}") for tau in range(TB)]
            for tau in range(TB):
                for n in range(8):
                    ps = psA.tile([128, 512], F32, space="PSUM")
                    for k in range(4):
                        nc.tensor.matmul(
                            out=ps[:B, :],
                            lhsT=xT[:, tau, k, :],
                            rhs=wihx_sb[:, k, ts(n, 512)],
                            start=(k == 0),
                            stop=(k == 3),
                        )
                    gblk = work.tile([B, 512], F16, tag="gxout")
                    nc.vector.tensor_add(out=gblk[:], in0=ps[:B, :], in1=tb_sb[:, ts(n, 512)])
                    nc.sync.dma_start(out=gx_in[tau][:, ts(n, 512)], in_=gblk[:])
                nc.gpsimd.collective_compute(
                    "AllGather",
                    mybir.AluOpType.bypass,
                    replica_groups=[list(range(NC))],
                    ins=[gx_in[tau][:].opt()],
                    outs=[ag_out[tau][:].opt()],
                )

            phase_stack.close()  # release phase_a + wstream SBUF
            res = stack.enter_context(tc.tile_pool(name="resident", bufs=1))

            # ---- resident weights for scan + projection (loads overlap AG#0 wait)
            whh_sb = res.tile([128, 8, G4], F16)
            nc.sync.dma_start(out=whh_sb[:], in_=whh.ap().rearrange("(k p) n -> p k n", p=128))
            wres = res.tile([128, 8, VS], F16)
            nc.sync.dma_start(out=wres[:], in_=wlin.ap().rearrange("(k p) n -> p k n", p=128))
            bias_bc = const.tile([128, VS], F16)
            nc.sync.dma_start(
                out=bias_bc[:],
                in_=bass.AP(tensor=blin.ap().tensor, offset=0, ap=[[0, 128], [1, VS]]),
            )

            # ---- scan state
            hT = res.tile([128, 8, 128], F16)
            nc.vector.memset(hT[:], 0.0)
            c_st = res.tile([B, H], F32)
            nc.vector.memset(c_st[:], 0.0)
            packed_dram = dram.tile([128, 8, p_pad], F16)

            # projection emission machinery: unit = (m, nchunk), 8 matmuls each
            proj_units = [(m, n) for m in range(m_tiles) for n in range(NCH)]
            emitted = [0]  # index into proj_units
            cur_lhs = [None, -1]  # tile, m

            def emit_proj_units(avail_rows, count):
                for _ in range(count):
                    if emitted[0] >= len(proj_units):
                        return
                    m, n = proj_units[emitted[0]]
                    if (m + 1) * 128 > avail_rows:
                        return
                    emitted[0] += 1
                    if cur_lhs[1] != m:
                        lh = lhsp.tile([128, 8, 128], F16, tag="lhs")
                        nc.sync.dma_start(out=lh[:], in_=packed_dram[:, :, ts(m, 128)])
                        cur_lhs[0], cur_lhs[1] = lh, m
                    lh = cur_lhs[0]
                    n0 = n * 512
                    nsz = min(512, VS - n0)
                    ps = psA.tile([128, 512], F32, space="PSUM")
                    for k in range(8):
                        nc.tensor.matmul(
                            out=ps[:, :nsz],
                            lhsT=lh[:, k, :],
                            rhs=wres[:, k, n0 : n0 + nsz],
                            start=(k == 0),
                            stop=(k == 7),
                        )
                    ost = ostage.tile([128, 512], F32, tag="ost")
                    nc.vector.tensor_add(
                        out=ost[:, :nsz], in0=ps[:, :nsz], in1=bias_bc[:, n0 : n0 + nsz]
                    )
                    nc.sync.dma_start(out=out[ts(m, 128), n0 : n0 + nsz], in_=ost[:, :nsz])

            for t in range(T):
                gxb = gxp.tile([B, G4], F16, tag="gxblk")
                nc.sync.dma_start(out=gxb[:], in_=ag_out[t // NC][t % NC, :, :])
                acts = {}
                for n in gate_order:
                    gt = gates.tile([B, 1024], F16, tag=f"gate{n}")
                    for hf in range(2):
                        ps = psA.tile([128, 512], F32, space="PSUM")
                        for k in range(8):
                            nc.tensor.matmul(
                                out=ps[:B, :],
                                lhsT=hT[:, k, :],
                                rhs=whh_sb[:, k, n * 1024 + hf * 512 : n * 1024 + (hf + 1) * 512],
                                start=(k == 0),
                                stop=(k == 7),
                            )
                        nc.vector.tensor_add(
                            out=gt[:, ts(hf, 512)],
                            in0=ps[:B, :],
                            in1=gxb[:, n * 1024 + hf * 512 : n * 1024 + (hf + 1) * 512],
                        )
                    nc.scalar.activation(gt[:], gt[:], gate_fn[n])
                    acts[n] = gt

                # projection fill work (uses rows packed by prior steps) sits in
                # the PE stream between the gate matmuls and the h transposes,
                # so TensorE stays busy while DVE/ACT run the c/h tail.
                emit_proj_units(off_t[t], 5 if t >= 2 else 0)

                h = stmp.tile([B, H], F16, tag="h")
                for hf in range(2):
                    sl = slice(hf * 512, (hf + 1) * 512)
                    ig = stmp.tile([B, 512], F32, tag=f"ig{hf}")
                    nc.vector.tensor_mul(out=ig[:], in0=acts[0][:, sl], in1=acts[2][:, sl])
                    fc = stmp.tile([B, 512], F32, tag=f"fc{hf}")
                    nc.vector.tensor_mul(out=fc[:], in0=acts[1][:, sl], in1=c_st[:, sl])
                    nc.vector.tensor_add(out=c_st[:, sl], in0=ig[:], in1=fc[:])
                    thc = stmp.tile([B, 512], F16, tag=f"thc{hf}")
                    nc.scalar.activation(thc[:], c_st[:, sl], AF.Tanh)
                    nc.vector.tensor_mul(out=h[:, sl], in0=acts[3][:, sl], in1=thc[:])
                    for k in range(4 * hf, 4 * hf + 4):
                        pt = psT.tile([128, 128], F16, space="PSUM")
                        nc.tensor.transpose(pt[:], h[:, ts(k, 128)], ident[:])
                        nc.vector.tensor_copy(out=hT[:, k, :], in_=pt[:])
                if n_t[t] > 0:
                    nc.sync.dma_start(
                        out=packed_dram[:, :, off_t[t] : off_t[t] + n_t[t]],
                        in_=hT[:, :, : n_t[t]],
                    )

            # ---- projection epilogue: whatever didn't fit in the scan
            emit_proj_units(p_pad, len(proj_units))

    nc.finalize()
    return nc


def kernel(features, tags, captions, lengths, W_embed, W_ih, W_hh, b_ih, b_hh, W_lin, b_lin):
    features = np.asarray(features, dtype=np.float32)
    tags = np.asarray(tags, dtype=np.float32)
    captions = np.asarray(captions)
    lengths = np.asarray(lengths)
    W_embed = np.asarray(W_embed, dtype=np.float32)
    W_ih = np.asarray(W_ih, dtype=np.float32)
    W_hh = np.asarray(W_hh, dtype=np.float32)
    b_ih = np.asarray(b_ih, dtype=np.float32)
    b_hh = np.asarray(b_hh, dtype=np.float32)
    W_lin = np.asarray(W_lin, dtype=np.float32)
    b_lin = np.asarray(b_lin, dtype=np.float32)

    # packing schedule (replicates reference pack_padded_sequence exactly)
    n_t = [int((lengths > t).sum()) for t in range(T)]
    off_t = np.concatenate([[0], np.cumsum(n_t)]).astype(np.int64)
    sum_len = int(off_t[-1])
    p_pad = ((sum_len + 127) // 128) * 128

    nc = _build(n_t, off_t, p_pad)

    emb_tab = np.concatenate([W_embed, features], axis=0).astype(np.float16)
    wihx = np.ascontiguousarray(W_ih[:, :E].T).astype(np.float16)
    wiht = np.zeros((5 * 128, G4), np.float16)
    wiht[:TAG] = W_ih[:, E:].T.astype(np.float16)
    wiht[TAG] = (b_ih + b_hh).astype(np.float16)
    tags_t = np.zeros((5 * 128, B), np.float16)
    tags_t[:TAG] = tags.T.astype(np.float16)
    tags_t[TAG] = 1.0
    whh = np.ascontiguousarray(W_hh.T).astype(np.float16)

    in_maps = []
    for c in range(NC):
        idx = np.empty((B, TB), np.int32)
        for tau in range(TB):
            tt = tau * NC + c  # interleaved: AllGather #tau carries steps [8*tau, 8*tau+8)
            if tt == 0:
                idx[:, tau] = V + np.arange(B)
            else:
                idx[:, tau] = captions[:, tt - 1].astype(np.int32)
        wlin_c = np.ascontiguousarray(W_lin[c * VS : (c + 1) * VS].T).astype(np.float16)
        blin_c = np.ascontiguousarray(b_lin[c * VS : (c + 1) * VS]).astype(np.float16).reshape(1, VS)
        in_maps.append(
            {
                "emb_tab": emb_tab,
                "idx": idx,
                "wihx": wihx,
                "wiht": wiht,
                "tags_t": tags_t,
                "whh": whh,
                "wlin": wlin_c,
                "blin": blin_c,
            }
        )

    res = run_bass_kernel_spmd(nc, in_maps, list(range(NC)))

    out = np.empty((sum_len, V), np.float32)
    for c in range(NC):
        out[:, c * VS : (c + 1) * VS] = res.results[c]["out"][:sum_len]
    return out



# revision 5
# speedup vs baseline: 1.2868x; 1.2868x over previous
"""Trainium2 Bass kernel for nn_DecoderRNN (LSTM decoder with tag-conditioned
inputs, packed-sequence output projection).

Strategy (8 NeuronCores, SPMD single program):
  - LSTM recurrence replicated full-batch on every core (B=128 fills the PE
    partition dim; per-step cross-core h exchange is latency-prohibitive).
    The h@W_hh matmul runs fp8e4m3 with DoubleRow perf mode (K=256 per pass),
    W_hh pre-scaled by 16 into e4m3's normal range, compensated in the
    PSUM->SBUF fused scale+add.
  - gx (x@W_ihx + tags@W_iht + bias): steps 0-7 computed locally just-in-time
    inside the scan (fp16, no collective on the critical path); steps 8-31
    sharded one-step-per-core and delivered by 3 AllGathers that hide behind
    the scan.
  - Output projection: vocab-sharded fp16, interleaved into the scan as
    packed-row tiles complete; fp16 output staging (upcast to fp32 on host).
"""

import sys

sys.path.insert(0, "/opt/trn_rl_repo")

import ml_dtypes
import numpy as np

import concourse.bass as bass
import concourse.mybir as mybir
import concourse.tile as tile
from concourse import bacc
from concourse.bass import ts
from concourse.bass_utils import run_bass_kernel_spmd
from concourse.masks import make_identity

B, L, E, H, V, TAG = 128, 31, 512, 1024, 30000, 512
T = L + 1
NC = 8
VS = V // NC          # vocab shard per core
G4 = 4 * H            # gate width
F8 = mybir.dt.float8e4
F16 = mybir.dt.float16
F32 = mybir.dt.float32
AF = mybir.ActivationFunctionType
ALU = mybir.AluOpType
DR = mybir.MatmulPerfMode.DoubleRow
NCH = (VS + 511) // 512   # projection vocab chunks per core
NJIT = 8                  # steps with locally-computed gx
GXB = 3                   # gx slot ring size
WSCALE = 16.0             # W_hh pre-scale into fp8 normal range

# chunk order: gate g (tanh) first, o (sigmoid) last; chunk ch -> gate ch//2
CH_ORDER = [4, 5, 0, 1, 2, 3, 6, 7]
GATE_FN = {0: AF.Sigmoid, 1: AF.Sigmoid, 2: AF.Tanh, 3: AF.Sigmoid}
EMIT_A, EMIT_B = 2, 3     # proj units after gates / after transposes


def _build(n_t, off_t, p_pad):
    nc = bacc.Bacc(None, target_bir_lowering=False)

    emb_tab = nc.declare_dram_parameter("emb_tab", [V + B, E], F16, isOutput=False)
    idx_in = nc.declare_dram_parameter("idx", [B, NJIT + 3], mybir.dt.int32, isOutput=False)
    wihx = nc.declare_dram_parameter("wihx", [128, 4, G4], F16, isOutput=False)
    wiht = nc.declare_dram_parameter("wiht", [128, 5, G4], F16, isOutput=False)
    tags_d = nc.declare_dram_parameter("tags_d", [128, 5, B], F16, isOutput=False)
    whh8_d = nc.declare_dram_parameter("whh8", [128, 8, G4], F8, isOutput=False)
    wres_d = nc.declare_dram_parameter("wres", [128, 8, VS], F16, isOutput=False)
    blin = nc.declare_dram_parameter("blin", [1, VS], F16, isOutput=False)
    out = nc.declare_dram_parameter("out", [p_pad, VS], F16, isOutput=True)

    m_tiles = p_pad // 128

    from contextlib import ExitStack

    with tile.TileContext(nc) as tc:
        stack = ExitStack()
        with stack:
            const = stack.enter_context(tc.tile_pool(name="const", bufs=1))
            psT = stack.enter_context(tc.tile_pool(name="psT", bufs=2, space="PSUM"))
            psA = stack.enter_context(tc.tile_pool(name="psA", bufs=3, space="PSUM"))
            dram = stack.enter_context(tc.tile_pool(name="dram", bufs=1, space="DRAM"))
            res = stack.enter_context(tc.tile_pool(name="res", bufs=1))
            gates = stack.enter_context(tc.tile_pool(name="gates", bufs=1))
            stmp = stack.enter_context(tc.tile_pool(name="stmp", bufs=1))
            pre = stack.enter_context(tc.tile_pool(name="pre", bufs=1))
            gxp = stack.enter_context(tc.tile_pool(name="gxp", bufs=GXB))
            lhsp = stack.enter_context(tc.tile_pool(name="lhsp", bufs=2))
            ostage = stack.enter_context(tc.tile_pool(name="ostage", bufs=2))
            phase_stack = ExitStack()
            gath = phase_stack.enter_context(tc.tile_pool(name="gath", bufs=2))
            tagp = phase_stack.enter_context(tc.tile_pool(name="tagp", bufs=1))
            wstr = phase_stack.enter_context(tc.tile_pool(name="wstr", bufs=1))

            # ---- dummy collective first: absorbs ncfw warmup
            d_in = dram.tile([1, 128], F32)
            d_out = dram.tile([NC, 128], F32)
            d_in_sb = const.tile([1, 128], F32)
            nc.vector.memset(d_in_sb[:], 0.0)
            nc.sync.dma_start(out=d_in[:], in_=d_in_sb[:])
            nc.gpsimd.collective_compute(
                "AllGather",
                ALU.bypass,
                replica_groups=[list(range(NC))],
                ins=[d_in[:].opt()],
                outs=[d_out[:].opt()],
            )

            # ---- all big weight DMAs issued up-front, in need-order
            idx_sb = const.tile([B, NJIT + 3], mybir.dt.int32)
            nc.sync.dma_start(out=idx_sb[:], in_=idx_in[:, :])
            tags_sb = tagp.tile([128, 5, B], F16)
            nc.sync.dma_start(out=tags_sb[:], in_=tags_d[:, :, :])
            wihx_sb = pre.tile([128, 4, G4], F16)
            nc.sync.dma_start(out=wihx_sb[:], in_=wihx[:, :, :])
            whh8_sb = res.tile([128, 8, G4], F8)
            nc.sync.dma_start(out=whh8_sb[:], in_=whh8_d[:, :, :])

            ident = const.tile([128, 128], F16)
            make_identity(nc, ident)

            # ---- tb = tags@W_iht^T + bias  (fp16 operands, fp32 accum)
            tb = pre.tile([B, G4], F16)
            for n in range(8):
                w = wstr.tile([128, 5, 512], F16, tag="wstream")
                nc.sync.dma_start(out=w[:], in_=wiht[:, :, ts(n, 512)])
                ps = psA.tile([128, 512], F32, space="PSUM")
                for k in range(5):
                    nc.tensor.matmul(
                        out=ps[:B, :],
                        lhsT=tags_sb[:, k, :],
                        rhs=w[:, k, :],
                        start=(k == 0),
                        stop=(k == 4),
                    )
                nc.vector.tensor_copy(out=tb[:, ts(n, 512)], in_=ps[:B, :])

            # ---- gathers + transposes: xT[t] for steps 0..7, then shard steps
            # idx col t<8 -> step t; col 8+i -> shard step 8*(i+1)+core
            xT = pre.tile([128, NJIT, 4, 128], F16)  # [p, t, kE, token]

            def gather_transpose(col, xdst):
                g = gath.tile([B, E], F16, tag="gath")
                nc.gpsimd.indirect_dma_start(
                    out=g[:],
                    out_offset=None,
                    in_=emb_tab[:],
                    in_offset=bass.IndirectOffsetOnAxis(ap=idx_sb[:, col : col + 1], axis=0),
                )
                for k in range(4):
                    pt = psT.tile([128, 128], F16, space="PSUM")
                    nc.tensor.transpose(pt[:], g[:, ts(k, 128)], ident[:])
                    nc.vector.tensor_copy(out=xdst[:, k, :], in_=pt[:])

            xTs = pre.tile([128, 3, 4, 128], F16)  # shard steps
            for i in range(3):
                gather_transpose(NJIT + i, xTs[:, i])
            for t in range(NJIT):
                gather_transpose(t, xT[:, t])

            # ---- sharded gx for steps 8..31 + AllGathers (hide behind scan)
            gx_in = [dram.tile([B, G4], F16, name=f"gx_in{i}") for i in range(3)]
            ag_out = [dram.tile([NC, B, G4], F16, name=f"ag_out{i}") for i in range(3)]
            for i in range(3):
                # stage in the gx ring (temporally disjoint from scan gx use)
                gsh = gxp.tile([B, G4], F16, tag="gx", name=f"gsh{i}")
                for n in range(8):
                    ps = psA.tile([128, 512], F32, space="PSUM")
                    for k in range(4):
                        nc.tensor.matmul(
                            out=ps[:B, :],
                            lhsT=xTs[:, i, k, :],
                            rhs=wihx_sb[:, k, ts(n, 512)],
                            start=(k == 0),
                            stop=(k == 3),
                        )
                    nc.vector.tensor_add(
                        out=gsh[:, ts(n, 512)], in0=ps[:B, :], in1=tb[:, ts(n, 512)]
                    )
                nc.sync.dma_start(out=gx_in[i][:, :], in_=gsh[:])
                nc.gpsimd.collective_compute(
                    "AllGather",
                    ALU.bypass,
                    replica_groups=[list(range(NC))],
                    ins=[gx_in[i][:].opt()],
                    outs=[ag_out[i][:].opt()],
                )

            phase_stack.close()  # release gath/tagp/wstr/gshp SBUF

            # ---- remaining resident loads (needed from first proj emission)
            wres_sb = res.tile([128, 8, VS], F16)
            nc.sync.dma_start(out=wres_sb[:], in_=wres_d[:, :, :])
            bias_bc = const.tile([128, VS], F16)
            nc.sync.dma_start(
                out=bias_bc[:],
                in_=bass.AP(tensor=blin.ap().tensor, offset=0, ap=[[0, 128], [1, VS]]),
            )

            # ---- scan state (no memsets needed: t=0 skips h/c reads)
            hT16 = res.tile([128, 8, 128], F16)
            hT8 = res.tile([128, 8, 128], F8)
            c_st = res.tile([B, H], F16)
            h_sb = stmp.tile([B, H], F16)
            packed_dram = dram.tile([128, 8, p_pad], F16)

            # ---- gx slot machinery: slot t%GXB holds gx for step t
            def make_gx_slot(t):
                gx = gxp.tile([B, G4], F16, tag="gx")
                if t < NJIT:
                    for n in range(8):
                        ps = psA.tile([128, 512], F32, space="PSUM")
                        for k in range(4):
                            nc.tensor.matmul(
                                out=ps[:B, :],
                                lhsT=xT[:, t, k, :],
                                rhs=wihx_sb[:, k, ts(n, 512)],
                                start=(k == 0),
                                stop=(k == 3),
                            )
                        nc.vector.tensor_add(
                            out=gx[:, ts(n, 512)], in0=ps[:B, :], in1=tb[:, ts(n, 512)]
                        )
                else:
                    tau, slot = t // 8 - 1, t % 8
                    nc.sync.dma_start(out=gx[:], in_=ag_out[tau][slot, :, :])
                return gx

            # ---- projection emission machinery
            proj_units = [(m, n) for m in range(m_tiles) for n in range(NCH)]
            emitted = [0]
            cur_lhs = [None, -1]

            def emit_proj_units(avail_rows, count):
                for _ in range(count):
                    if emitted[0] >= len(proj_units):
                        return
                    m, n = proj_units[emitted[0]]
                    if (m + 1) * 128 > avail_rows:
                        return
                    emitted[0] += 1
                    if cur_lhs[1] != m:
                        lh = lhsp.tile([128, 8, 128], F16, tag="lhs")
                        nc.sync.dma_start(out=lh[:], in_=packed_dram[:, :, ts(m, 128)])
                        cur_lhs[0], cur_lhs[1] = lh, m
                    lh = cur_lhs[0]
                    n0 = n * 512
                    nsz = min(512, VS - n0)
                    ps = psA.tile([128, 512], F32, space="PSUM")
                    for k in range(8):
                        nc.tensor.matmul(
                            out=ps[:, :nsz],
                            lhsT=lh[:, k, :],
                            rhs=wres_sb[:, k, n0 : n0 + nsz],
                            start=(k == 0),
                            stop=(k == 7),
                        )
                    ost = ostage.tile([128, 512], F16, tag="ost")
                    nc.vector.tensor_add(
                        out=ost[:, :nsz], in0=ps[:, :nsz], in1=bias_bc[:, n0 : n0 + nsz]
                    )
                    nc.sync.dma_start(out=out[ts(m, 128), n0 : n0 + nsz], in_=ost[:, :nsz])

            # gx for steps 0 and 1 produced before the scan
            gx_slots = [None] * GXB
            for t in range(min(2, T)):
                gx_slots[t % GXB] = make_gx_slot(t)

            # ---- the scan
            for t in range(T):
                # produce gx for step t+2 (JIT matmuls or AG-delivered DMA)
                if t + 2 < T:
                    gx_slots[(t + 2) % GXB] = make_gx_slot(t + 2)
                gxb = gx_slots[t % GXB]

                acts = {}
                for ch in CH_ORDER:
                    g, hf = ch // 2, ch % 2
                    if g not in acts:
                        acts[g] = gates.tile([B, 1024], F16, tag=f"gate{g}", name=f"gate{g}")
                    gt = acts[g]
                    if t == 0:
                        nc.scalar.activation(
                            gt[:, ts(hf, 512)], gxb[:, ts(ch, 512)], GATE_FN[g]
                        )
                        continue
                    ps = psA.tile([128, 512], F32, space="PSUM")
                    for j in range(4):
                        nc.tensor.matmul(
                            out=ps[:B, :],
                            lhsT=hT8[:, 2 * j : 2 * j + 2, :],
                            rhs=whh8_sb[:, 2 * j : 2 * j + 2, ts(ch, 512)],
                            start=(j == 0),
                            stop=(j == 3),
                            perf_mode=DR,
                        )
                    nc.vector.scalar_tensor_tensor(
                        out=gt[:, ts(hf, 512)],
                        in0=ps[:B, :],
                        scalar=1.0 / WSCALE,
                        in1=gxb[:, ts(ch, 512)],
                        op0=ALU.mult,
                        op1=ALU.add,
                    )
                    nc.scalar.activation(gt[:, ts(hf, 512)], gt[:, ts(hf, 512)], GATE_FN[g])

                # proj fill: part A sits against the act/DVE tail
                emit_proj_units(off_t[t], EMIT_A if t >= 2 else 0)

                for hf in range(2):
                    sl = ts(hf, 512)
                    ig = stmp.tile([B, 512], F16, tag="ig")
                    nc.vector.tensor_mul(out=ig[:], in0=acts[0][:, sl], in1=acts[2][:, sl])
                    if t == 0:
                        nc.vector.tensor_copy(out=c_st[:, sl], in_=ig[:])
                    else:
                        fc = stmp.tile([B, 512], F16, tag="fc")
                        nc.vector.tensor_mul(out=fc[:], in0=acts[1][:, sl], in1=c_st[:, sl])
                        nc.vector.tensor_add(out=c_st[:, sl], in0=ig[:], in1=fc[:])
                    thc = stmp.tile([B, 512], F16, tag="thc")
                    nc.scalar.activation(thc[:], c_st[:, sl], AF.Tanh)
                    nc.vector.tensor_mul(out=h_sb[:, sl], in0=acts[3][:, sl], in1=thc[:])
                    for k in range(4 * hf, 4 * hf + 4):
                        pt = psT.tile([128, 128], F16, space="PSUM")
                        nc.tensor.transpose(pt[:], h_sb[:, ts(k, 128)], ident[:])
                        nc.vector.tensor_copy(out=hT16[:, k, :], in_=pt[:])
                        nc.vector.tensor_copy(out=hT8[:, k, :], in_=pt[:])
                if n_t[t] > 0:
                    nc.sync.dma_start(
                        out=packed_dram[:, :, off_t[t] : off_t[t] + n_t[t]],
                        in_=hT16[:, :, : n_t[t]],
                    )
                emit_proj_units(off_t[t], EMIT_B if t >= 2 else 0)

            # ---- projection epilogue
            emit_proj_units(p_pad, len(proj_units))

    nc.finalize()
    return nc


def kernel(features, tags, captions, lengths, W_embed, W_ih, W_hh, b_ih, b_hh, W_lin, b_lin):
    features = np.asarray(features, dtype=np.float32)
    tags = np.asarray(tags, dtype=np.float32)
    captions = np.asarray(captions)
    lengths = np.asarray(lengths)
    W_embed = np.asarray(W_embed, dtype=np.float32)
    W_ih = np.asarray(W_ih, dtype=np.float32)
    W_hh = np.asarray(W_hh, dtype=np.float32)
    b_ih = np.asarray(b_ih, dtype=np.float32)
    b_hh = np.asarray(b_hh, dtype=np.float32)
    W_lin = np.asarray(W_lin, dtype=np.float32)
    b_lin = np.asarray(b_lin, dtype=np.float32)

    # packing schedule (replicates reference pack_padded_sequence exactly)
    n_t = [int((lengths > t).sum()) for t in range(T)]
    off_t = np.concatenate([[0], np.cumsum(n_t)]).astype(np.int64)
    sum_len = int(off_t[-1])
    p_pad = ((sum_len + 127) // 128) * 128

    nc = _build(n_t, off_t, p_pad)

    def kpn(a, kt):  # [K, N] -> [128, kt, N]
        return np.ascontiguousarray(
            a.reshape(kt, 128, -1).transpose(1, 0, 2)
        )

    emb_tab = np.concatenate([W_embed, features], axis=0).astype(np.float16)
    wihx = kpn(np.ascontiguousarray(W_ih[:, :E].T), 4).astype(np.float16)
    wiht_f = np.zeros((5 * 128, G4), np.float32)
    wiht_f[:TAG] = W_ih[:, E:].T
    wiht_f[TAG] = b_ih + b_hh
    wiht = kpn(wiht_f, 5).astype(np.float16)
    tags_f = np.zeros((5 * 128, B), np.float32)
    tags_f[:TAG] = tags.T
    tags_f[TAG] = 1.0
    tags_d = kpn(tags_f, 5).astype(np.float16)
    whh8 = kpn(np.ascontiguousarray((W_hh * WSCALE).T), 8).astype(ml_dtypes.float8_e4m3fn)

    in_maps = []
    for cid in range(NC):
        idx = np.empty((B, NJIT + 3), np.int32)
        idx[:, 0] = V + np.arange(B)
        for t in range(1, NJIT):
            idx[:, t] = captions[:, t - 1].astype(np.int32)
        for i in range(3):
            step = 8 * (i + 1) + cid
            idx[:, NJIT + i] = captions[:, step - 1].astype(np.int32)
        wres = kpn(
            np.ascontiguousarray(W_lin[cid * VS : (cid + 1) * VS].T), 8
        ).astype(np.float16)
        blin_c = (
            np.ascontiguousarray(b_lin[cid * VS : (cid + 1) * VS])
            .astype(np.float16)
            .reshape(1, VS)
        )
        in_maps.append(
            {
                "emb_tab": emb_tab,
                "idx": idx,
                "wihx": wihx,
                "wiht": wiht,
                "tags_d": tags_d,
                "whh8": whh8,
                "wres": wres,
                "blin": blin_c,
            }
        )

    res = run_bass_kernel_spmd(nc, in_maps, list(range(NC)))

    out = np.empty((sum_len, V), np.float32)
    for cid in range(NC):
        out[:, cid * VS : (cid + 1) * VS] = res.results[cid]["out"][:sum_len].astype(
            np.float32
        )
    return out


# revision 8
# speedup vs baseline: 1.3466x; 1.0465x over previous
"""Trainium2 Bass kernel for nn_DecoderRNN (LSTM decoder with tag-conditioned
inputs, packed-sequence output projection).

Strategy (8 NeuronCores, SPMD single program):
  - LSTM recurrence replicated full-batch on every core (B=128 fills the PE
    partition dim; per-step cross-core h exchange is latency-prohibitive).
    The h@W_hh matmul runs fp8e4m3 with DoubleRow perf mode (K=256 per pass);
    W_hh and W_ihx are pre-scaled by 16 into e4m3's normal range, compensated
    in the fused PSUM->SBUF scale+add.
  - gx (x@W_ihx + tags@W_iht + bias): steps 0-7 fused directly into the gate
    PSUM accumulation (fp16 x-matmuls share the group with fp8 h-matmuls; no
    collective, no staging on the critical path); steps 8-31 sharded
    one-step-per-core and delivered by 3 AllGathers that hide behind the scan.
  - Output projection: vocab-sharded fp16, interleaved into the scan as
    packed-row tiles complete; fp16 output staging (upcast to fp32 on host).
"""

import sys

sys.path.insert(0, "/opt/trn_rl_repo")

import ml_dtypes
import numpy as np

import concourse.bass as bass
import concourse.mybir as mybir
import concourse.tile as tile
from concourse import bacc
from concourse.bass import ts
from concourse.bass_utils import run_bass_kernel_spmd
from concourse.masks import make_identity

B, L, E, H, V, TAG = 128, 31, 512, 1024, 30000, 512
T = L + 1
NC = 8
VS = V // NC          # vocab shard per core
G4 = 4 * H            # gate width
F8 = mybir.dt.float8e4
F16 = mybir.dt.float16
F32 = mybir.dt.float32
AF = mybir.ActivationFunctionType
ALU = mybir.AluOpType
DR = mybir.MatmulPerfMode.DoubleRow
NCH = (VS + 511) // 512   # projection vocab chunks per core
NJIT = 8                  # steps with locally-computed (fused) gx
GXB = 3                   # AG-delivered gx slot ring size
WSCALE = 16.0             # W_hh / W_ihx pre-scale into fp8 normal range

# chunk order: gate g (tanh) first, o (sigmoid) last; chunk ch -> gate ch//2
CH_ORDER = [4, 5, 0, 1, 2, 3, 6, 7]
GATE_FN = {0: AF.Sigmoid, 1: AF.Sigmoid, 2: AF.Tanh, 3: AF.Sigmoid}
EMIT_A, EMIT_B = 2, 3     # proj units after gates / after transposes


def _build(n_t, off_t, p_pad, sum_len):
    nc = bacc.Bacc(None, target_bir_lowering=False)

    emb_tab = nc.declare_dram_parameter("emb_tab", [V + B, E], F16, isOutput=False)
    idx_in = nc.declare_dram_parameter("idx", [B, NJIT + 3], mybir.dt.int32, isOutput=False)
    wihx = nc.declare_dram_parameter("wihx", [128, 4, G4], F16, isOutput=False)
    wiht = nc.declare_dram_parameter("wiht", [128, 5, G4], F16, isOutput=False)
    tags_d = nc.declare_dram_parameter("tags_d", [128, 5, B], F16, isOutput=False)
    whh8_d = nc.declare_dram_parameter("whh8", [128, 8, G4], F8, isOutput=False)
    wres_d = nc.declare_dram_parameter("wres", [128, 8, VS], F16, isOutput=False)
    blin = nc.declare_dram_parameter("blin", [1, VS], F16, isOutput=False)
    out = nc.declare_dram_parameter("out", [p_pad, VS], F16, isOutput=True)

    m_tiles = p_pad // 128

    from contextlib import ExitStack

    with tile.TileContext(nc) as tc:
        stack = ExitStack()
        with stack:
            const = stack.enter_context(tc.tile_pool(name="const", bufs=1))
            psT = stack.enter_context(tc.tile_pool(name="psT", bufs=2, space="PSUM"))
            psA = stack.enter_context(tc.tile_pool(name="psA", bufs=3, space="PSUM"))
            dram = stack.enter_context(tc.tile_pool(name="dram", bufs=1, space="DRAM"))
            res = stack.enter_context(tc.tile_pool(name="res", bufs=1))
            gates = stack.enter_context(tc.tile_pool(name="gates", bufs=1))
            stmp = stack.enter_context(tc.tile_pool(name="stmp", bufs=1))
            pre = stack.enter_context(tc.tile_pool(name="pre", bufs=1))
            gxp = stack.enter_context(tc.tile_pool(name="gxp", bufs=GXB))
            lhsp = stack.enter_context(tc.tile_pool(name="lhsp", bufs=2))
            ostage = stack.enter_context(tc.tile_pool(name="ostage", bufs=2))
            phase_stack = ExitStack()
            gath = phase_stack.enter_context(tc.tile_pool(name="gath", bufs=2))
            tagp = phase_stack.enter_context(tc.tile_pool(name="tagp", bufs=1))
            wstr = phase_stack.enter_context(tc.tile_pool(name="wstr", bufs=1))

            # ---- dummy collective first: absorbs ncfw warmup
            d_in = dram.tile([1, 128], F32)
            d_out = dram.tile([NC, 128], F32)
            d_in_sb = const.tile([1, 128], F32)
            nc.vector.memset(d_in_sb[:], 0.0)
            nc.sync.dma_start(out=d_in[:], in_=d_in_sb[:])
            nc.gpsimd.collective_compute(
                "AllGather",
                ALU.bypass,
                replica_groups=[list(range(NC))],
                ins=[d_in[:].opt()],
                outs=[d_out[:].opt()],
            )

            # ---- all big weight DMAs issued up-front, in need-order
            idx_sb = const.tile([B, NJIT + 3], mybir.dt.int32)
            nc.sync.dma_start(out=idx_sb[:], in_=idx_in[:, :])
            tags_sb = tagp.tile([128, 5, B], F16)
            nc.sync.dma_start(out=tags_sb[:], in_=tags_d[:, :, :])
            wihx_sb = pre.tile([128, 4, G4], F16)
            nc.sync.dma_start(out=wihx_sb[:], in_=wihx[:, :, :])
            whh8_sb = res.tile([128, 8, G4], F8)
            nc.sync.dma_start(out=whh8_sb[:], in_=whh8_d[:, :, :])

            ident = const.tile([128, 128], F16)
            make_identity(nc, ident)

            # ---- gathers + transposes during the weight-DMA window
            # idx col t<8 -> step t; col 8+i -> shard step 8*(i+1)+core
            xT = pre.tile([128, NJIT, 4, 128], F16)   # [p, t, kE, token]
            xTs = pre.tile([128, 3, 4, 128], F16)     # shard steps

            def gather_transpose(col, xdst):
                g = gath.tile([B, E], F16, tag="gath")
                nc.gpsimd.indirect_dma_start(
                    out=g[:],
                    out_offset=None,
                    in_=emb_tab[:],
                    in_offset=bass.IndirectOffsetOnAxis(ap=idx_sb[:, col : col + 1], axis=0),
                )
                for k in range(4):
                    pt = psT.tile([128, 128], F16, space="PSUM")
                    nc.tensor.transpose(pt[:], g[:, ts(k, 128)], ident[:])
                    nc.vector.tensor_copy(out=xdst[:, k, :], in_=pt[:])

            for t in range(NJIT):
                gather_transpose(t, xT[:, t])
            for i in range(3):
                gather_transpose(NJIT + i, xTs[:, i])

            # ---- tb = tags@W_iht^T + bias  (fp16 operands, fp32 accum)
            tb = pre.tile([B, G4], F16)
            for n in range(8):
                w = wstr.tile([128, 5, 512], F16, tag="wstream")
                nc.sync.dma_start(out=w[:], in_=wiht[:, :, ts(n, 512)])
                ps = psA.tile([128, 512], F32, space="PSUM")
                for k in range(5):
                    nc.tensor.matmul(
                        out=ps[:B, :],
                        lhsT=tags_sb[:, k, :],
                        rhs=w[:, k, :],
                        start=(k == 0),
                        stop=(k == 4),
                    )
                nc.vector.tensor_copy(out=tb[:, ts(n, 512)], in_=ps[:B, :])

            phase_stack.close()  # release gath/tagp/wstr SBUF

            # ---- remaining resident loads (needed from first proj emission)
            wres_sb = res.tile([128, 8, VS], F16)
            nc.sync.dma_start(out=wres_sb[:], in_=wres_d[:, :, :])
            bias_bc = const.tile([128, VS], F16)
            nc.sync.dma_start(
                out=bias_bc[:],
                in_=bass.AP(tensor=blin.ap().tensor, offset=0, ap=[[0, 128], [1, VS]]),
            )

            # ---- scan state (no memsets needed: t=0 skips h/c reads)
            hT16 = res.tile([128, 8, 128], F16)
            hT8 = res.tile([128, 8, 128], F8)
            c_st = res.tile([B, H], F16)
            h_sb = stmp.tile([B, H], F16)
            packed_dram = dram.tile([128, 8, p_pad], F16)

            gx_in = [dram.tile([B, G4], F16, name=f"gx_in{i}") for i in range(3)]
            ag_out = [dram.tile([NC, B, G4], F16, name=f"ag_out{i}") for i in range(3)]

            def emit_shard(i):
                # this core's contribution to AG#i (step 8*(i+1)+core), then AG
                gsh = gxp.tile([B, G4], F16, tag="gx", name=f"gsh{i}")
                for n in range(8):
                    ps = psA.tile([128, 512], F32, space="PSUM")
                    for k in range(4):
                        nc.tensor.matmul(
                            out=ps[:B, :],
                            lhsT=xTs[:, i, k, :],
                            rhs=wihx_sb[:, k, ts(n, 512)],
                            start=(k == 0),
                            stop=(k == 3),
                        )
                    nc.vector.scalar_tensor_tensor(
                        out=gsh[:, ts(n, 512)],
                        in0=ps[:B, :],
                        scalar=1.0 / WSCALE,
                        in1=tb[:, ts(n, 512)],
                        op0=ALU.mult,
                        op1=ALU.add,
                    )
                nc.sync.dma_start(out=gx_in[i][:, :], in_=gsh[:])
                nc.gpsimd.collective_compute(
                    "AllGather",
                    ALU.bypass,
                    replica_groups=[list(range(NC))],
                    ins=[gx_in[i][:].opt()],
                    outs=[ag_out[i][:].opt()],
                )

            # ---- projection emission machinery
            proj_units = [(m, n) for m in range(m_tiles) for n in range(NCH)]
            emitted = [0]
            cur_lhs = [None, -1]

            def emit_proj_units(avail_rows, count):
                for _ in range(count):
                    if emitted[0] >= len(proj_units):
                        return
                    m, n = proj_units[emitted[0]]
                    if min((m + 1) * 128, sum_len) > avail_rows:
                        return
                    emitted[0] += 1
                    if cur_lhs[1] != m:
                        lh = lhsp.tile([128, 8, 128], F16, tag="lhs")
                        nc.sync.dma_start(out=lh[:], in_=packed_dram[:, :, ts(m, 128)])
                        cur_lhs[0], cur_lhs[1] = lh, m
                    lh = cur_lhs[0]
                    n0 = n * 512
                    nsz = min(512, VS - n0)
                    ps = psA.tile([128, 512], F32, space="PSUM")
                    for k in range(8):
                        nc.tensor.matmul(
                            out=ps[:, :nsz],
                            lhsT=lh[:, k, :],
                            rhs=wres_sb[:, k, n0 : n0 + nsz],
                            start=(k == 0),
                            stop=(k == 7),
                        )
                    ost = ostage.tile([128, 512], F16, tag="ost")
                    nc.vector.tensor_add(
                        out=ost[:, :nsz], in0=ps[:, :nsz], in1=bias_bc[:, n0 : n0 + nsz]
                    )
                    nc.sync.dma_start(out=out[ts(m, 128), n0 : n0 + nsz], in_=ost[:, :nsz])

            # AG-delivered gx slots for steps 8..T-1
            gx_slots = {}

            def fetch_gx(t):
                gx = gxp.tile([B, G4], F16, tag="gx", name=f"gx{t}")
                tau, slot = t // 8 - 1, t % 8
                nc.sync.dma_start(out=gx[:], in_=ag_out[tau][slot, :, :])
                gx_slots[t] = gx

            # ---- the scan
            for t in range(T):
                if t + 2 >= NJIT and t + 2 < T:
                    fetch_gx(t + 2)
                gxb = gx_slots.pop(t, None)  # None for fused (JIT) steps

                acts = {}
                for ch in CH_ORDER:
                    g, hf = ch // 2, ch % 2
                    if g not in acts:
                        acts[g] = gates.tile([B, 1024], F16, tag=f"gate{g}", name=f"gate{g}")
                    gt = acts[g]
                    ps = psA.tile([128, 512], F32, space="PSUM")
                    nmm = (4 if t < NJIT else 0) + (4 if t > 0 else 0)
                    i_mm = 0
                    if t < NJIT:  # fused x-part (fp16, 16x scale)
                        for k in range(4):
                            nc.tensor.matmul(
                                out=ps[:B, :],
                                lhsT=xT[:, t, k, :],
                                rhs=wihx_sb[:, k, ts(ch, 512)],
                                start=(i_mm == 0),
                                stop=(i_mm == nmm - 1),
                            )
                            i_mm += 1
                    if t > 0:  # recurrence (fp8 DoubleRow, 16x scale)
                        for j in range(4):
                            nc.tensor.matmul(
                                out=ps[:B, :],
                                lhsT=hT8[:, 2 * j : 2 * j + 2, :],
                                rhs=whh8_sb[:, 2 * j : 2 * j + 2, ts(ch, 512)],
                                start=(i_mm == 0),
                                stop=(i_mm == nmm - 1),
                                perf_mode=DR,
                            )
                            i_mm += 1
                    nc.vector.scalar_tensor_tensor(
                        out=gt[:, ts(hf, 512)],
                        in0=ps[:B, :],
                        scalar=1.0 / WSCALE,
                        in1=(tb if gxb is None else gxb)[:, ts(ch, 512)],
                        op0=ALU.mult,
                        op1=ALU.add,
                    )
                    nc.scalar.activation(gt[:, ts(hf, 512)], gt[:, ts(hf, 512)], GATE_FN[g])

                # proj fill: part A sits against the act/DVE tail
                emit_proj_units(off_t[t], EMIT_A if t >= 2 else 0)

                for hf in range(2):
                    sl = ts(hf, 512)
                    ig = stmp.tile([B, 512], F16, tag="ig")
                    nc.vector.tensor_mul(out=ig[:], in0=acts[0][:, sl], in1=acts[2][:, sl])
                    if t == 0:
                        nc.vector.tensor_copy(out=c_st[:, sl], in_=ig[:])
                    else:
                        fc = stmp.tile([B, 512], F16, tag="fc")
                        nc.vector.tensor_mul(out=fc[:], in0=acts[1][:, sl], in1=c_st[:, sl])
                        nc.vector.tensor_add(out=c_st[:, sl], in0=ig[:], in1=fc[:])
                    thc = stmp.tile([B, 512], F16, tag="thc")
                    nc.scalar.activation(thc[:], c_st[:, sl], AF.Tanh)
                    nc.vector.tensor_mul(out=h_sb[:, sl], in0=acts[3][:, sl], in1=thc[:])
                    for k in range(4 * hf, 4 * hf + 4):
                        pt = psT.tile([128, 128], F16, space="PSUM")
                        nc.tensor.transpose(pt[:], h_sb[:, ts(k, 128)], ident[:])
                        nc.vector.tensor_copy(out=hT16[:, k, :], in_=pt[:])
                        nc.vector.tensor_copy(out=hT8[:, k, :], in_=pt[:])
                if n_t[t] > 0:
                    nc.sync.dma_start(
                        out=packed_dram[:, :, off_t[t] : off_t[t] + n_t[t]],
                        in_=hT16[:, :, : n_t[t]],
                    )
                if t < 3:
                    emit_shard(t)  # sharded gx + AllGather for steps 8..31
                emit_proj_units(off_t[t], EMIT_B if t >= 2 else 0)

            # ---- projection epilogue
            emit_proj_units(p_pad, len(proj_units))

    nc.finalize()
    return nc


def kernel(features, tags, captions, lengths, W_embed, W_ih, W_hh, b_ih, b_hh, W_lin, b_lin):
    features = np.asarray(features, dtype=np.float32)
    tags = np.asarray(tags, dtype=np.float32)
    captions = np.asarray(captions)
    lengths = np.asarray(lengths)
    W_embed = np.asarray(W_embed, dtype=np.float32)
    W_ih = np.asarray(W_ih, dtype=np.float32)
    W_hh = np.asarray(W_hh, dtype=np.float32)
    b_ih = np.asarray(b_ih, dtype=np.float32)
    b_hh = np.asarray(b_hh, dtype=np.float32)
    W_lin = np.asarray(W_lin, dtype=np.float32)
    b_lin = np.asarray(b_lin, dtype=np.float32)

    # packing schedule (replicates reference pack_padded_sequence exactly)
    n_t = [int((lengths > t).sum()) for t in range(T)]
    off_t = np.concatenate([[0], np.cumsum(n_t)]).astype(np.int64)
    sum_len = int(off_t[-1])
    p_pad = ((sum_len + 127) // 128) * 128

    nc = _build(n_t, off_t, p_pad, sum_len)

    def kpn(a, kt):  # [K, N] -> [128, kt, N]
        return np.ascontiguousarray(a.reshape(kt, 128, -1).transpose(1, 0, 2))

    emb_tab = np.concatenate([W_embed, features], axis=0).astype(np.float16)
    wihx = kpn(np.ascontiguousarray((W_ih[:, :E] * WSCALE).T), 4).astype(np.float16)
    wiht_f = np.zeros((5 * 128, G4), np.float32)
    wiht_f[:TAG] = W_ih[:, E:].T
    wiht_f[TAG] = b_ih + b_hh
    wiht = kpn(wiht_f, 5).astype(np.float16)
    tags_f = np.zeros((5 * 128, B), np.float32)
    tags_f[:TAG] = tags.T
    tags_f[TAG] = 1.0
    tags_d = kpn(tags_f, 5).astype(np.float16)
    whh8 = kpn(np.ascontiguousarray((W_hh * WSCALE).T), 8).astype(ml_dtypes.float8_e4m3fn)

    in_maps = []
    for cid in range(NC):
        idx = np.empty((B, NJIT + 3), np.int32)
        idx[:, 0] = V + np.arange(B)
        for t in range(1, NJIT):
            idx[:, t] = captions[:, t - 1].astype(np.int32)
        for i in range(3):
            step = 8 * (i + 1) + cid
            idx[:, NJIT + i] = captions[:, step - 1].astype(np.int32)
        wres = kpn(
            np.ascontiguousarray(W_lin[cid * VS : (cid + 1) * VS].T), 8
        ).astype(np.float16)
        blin_c = (
            np.ascontiguousarray(b_lin[cid * VS : (cid + 1) * VS])
            .astype(np.float16)
            .reshape(1, VS)
        )
        in_maps.append(
            {
                "emb_tab": emb_tab,
                "idx": idx,
                "wihx": wihx,
                "wiht": wiht,
                "tags_d": tags_d,
                "whh8": whh8,
                "wres": wres,
                "blin": blin_c,
            }
        )

    res = run_bass_kernel_spmd(nc, in_maps, list(range(NC)))

    out = np.empty((sum_len, V), np.float32)
    for cid in range(NC):
        out[:, cid * VS : (cid + 1) * VS] = res.results[cid]["out"][:sum_len].astype(
            np.float32
        )
    return out
